# revision 4
# baseline (speedup 1.0000x reference)
import sys
sys.path.insert(0, '/opt/trn_rl_repo')
import numpy as np
import ml_dtypes

from concourse import bass, mybir, bacc
from concourse.tile import TileContext
from concourse.masks import make_identity
from concourse import bass_utils

# ---- problem constants (hardcoded) ----
D = 64
H = 8
L = 5
NP = 4            # points
DH = 8
NQ = 20000
B = 2
LIN = 45109
SS = [(184, 184), (92, 92), (46, 46), (23, 23), (12, 12)]   # (Hl, Wl)
LSI = [0, 33856, 42320, 44436, 44965]
STRIDE = 187                       # padded row stride in cells (>= Wl+3)
ROWS = [h + 3 for (h, w) in SS]    # rows incl. 1-top border + 2 bottom pad
CL = [r * STRIDE for r in ROWS]    # raw cells per level
CLP = [((c + 1023) // 1024) * 1024 for c in CL]   # level cells padded to groups
LBASE = [0]
for c in CLP[:-1]:
    LBASE.append(LBASE[-1] + c)
NCELL = sum(CLP)                   # 71680
NGRP = NCELL // 1024               # 70
NR = NCELL // 128                  # 560 rows per partition per head
OVER = 188                         # pair-row overlap read per partition
NQP = 5120                         # queries per core (padded)
NT = NQP // 128                    # 40 query tiles
NS = H * L * NP                    # 160 sample slots per query
FP32 = mybir.dt.float32
BF16 = mybir.dt.bfloat16
INT32 = mybir.dt.int32
AX = mybir.AluOpType
AF = mybir.ActivationFunctionType


def _build_tables():
    # per-slot (j = h*20 + l*4 + p) constant rows
    t_wl = np.zeros(NS, np.float32)
    t_hl = np.zeros(NS, np.float32)
    t_cxhi = np.zeros(NS, np.float32)
    t_cyhi = np.zeros(NS, np.float32)
    t_base = np.zeros(NS, np.float32)
    for h in range(H):
        for l in range(L):
            hl, wl = SS[l]
            for p in range(NP):
                j = h * (L * NP) + l * NP + p
                t_wl[j] = wl
                t_hl[j] = hl
                t_cxhi[j] = wl + 1
                t_cyhi[j] = hl + 1
                t_base[j] = h * NCELL + LBASE[l]
    return t_wl, t_hl, t_cxhi, t_cyhi, t_base


def build_program(nt=NT, gcols=80, no_gather=False, no_vpipe=False,
                  no_blend=False, no_tail=False, no_b=False, vmm1=False,
                  fxe=False, qhb=2, qbb=2, qgb=2):
    nc = bacc.Bacc()
    dt = nc.dram_tensor
    vT = dt("vT", (D + 1, NCELL), BF16, kind="ExternalInput")
    qsT = dt("qsT", (D, NQP), BF16, kind="ExternalInput")
    qf = dt("qf", (NQP, D), FP32, kind="ExternalInput")
    ref = dt("ref", (NQP, 2), FP32, kind="ExternalInput")
    Wv = dt("Wv", (D + 1, D), BF16, kind="ExternalInput")
    Wo = dt("Wo", (D, H * L * NP * 2), BF16, kind="ExternalInput")
    Wa = dt("Wa", (D, NS), BF16, kind="ExternalInput")
    Wout = dt("Wout", (D + 1, D), BF16, kind="ExternalInput")
    W1 = dt("W1", (D + 1, 1024), BF16, kind="ExternalInput")
    W2 = dt("W2", (128, 8 * D), BF16, kind="ExternalInput")
    bo_r = dt("bo_r", (128, 320), FP32, kind="ExternalInput")
    ba_r = dt("ba_r", (128, NS), FP32, kind="ExternalInput")
    g1_r = dt("g1_r", (128, D), FP32, kind="ExternalInput")
    b1_r = dt("b1_r", (128, D), FP32, kind="ExternalInput")
    g2_r = dt("g2_r", (128, D), FP32, kind="ExternalInput")
    b2_r = dt("b2_r", (128, D), FP32, kind="ExternalInput")
    bff2_r = dt("bff2_r", (128, D), FP32, kind="ExternalInput")
    t_wh = dt("t_wh", (128, 2 * NS), FP32, kind="ExternalInput")
    t_cxy = dt("t_cxy", (128, 2 * NS), FP32, kind="ExternalInput")
    t_base = dt("t_base", (128, NS), FP32, kind="ExternalInput")
    OUT = dt("out", (NQP, D), FP32, kind="ExternalOutput")
    # per-head table: row (h*NCELL + cell) holds that head's 8 dims of a cell
    TBL = dt("tbl", ((H * NCELL + OVER) * DH,), BF16, kind="Internal")
    # pair-row table: row (h*NCELL + cell) holds 8 dims of cell and cell+STRIDE
    T3 = dt("t3", (H * NCELL * 2 * DH,), BF16, kind="Internal")
    T3v = T3[:].rearrange("(c d) -> c d", d=2 * DH)

    with TileContext(nc) as tc:
        with tc.tile_pool(name="const", bufs=1) as cp:
            def ld(src, shape, dtype=FP32):
                t = cp.tile(shape, dtype, tag=src.name + "_sb")
                nc.sync.dma_start(t[:], src[:])
                return t
            Wv_sb = ld(Wv, [D + 1, D], BF16)
            Wo_sb = ld(Wo, [D, 320], BF16)
            Wa_sb = ld(Wa, [D, NS], BF16)
            Wout_sb = ld(Wout, [D + 1, D], BF16)
            W1_sb = ld(W1, [D + 1, 1024], BF16)
            W2_sb = ld(W2, [128, 8 * D], BF16)
            bo_sb = ld(bo_r, [128, 320])
            ba_sb = ld(ba_r, [128, NS])
            g1_sb = ld(g1_r, [128, D])
            b1_sb = ld(b1_r, [128, D])
            g2_sb = ld(g2_r, [128, D])
            b2_sb = ld(b2_r, [128, D])
            bff2_sb = ld(bff2_r, [128, D])
            twh_sb = ld(t_wh, [128, 2 * NS])
            tcxy_sb = ld(t_cxy, [128, 2 * NS])
            tbase_sb = ld(t_base, [128, NS])
            eps_sb = cp.tile([128, 1], FP32, tag="eps")
            nc.vector.memset(eps_sb[:], 1e-5)
            ident = cp.tile([128, 128], FP32, tag="ident")
            make_identity(nc, ident[:])
            # preload the one ACT func set that covers every func we use
            # (copy/exp/ln/relu: natural_log_exp_and_others, id 6) so the
            # table-load pass never inserts another reload
            nc.scalar.add_instruction(mybir.InstLoadActFuncSet(
                name=nc.scalar.bass.get_next_instruction_name(),
                act_func_set_id=6, ins=[], outs=[]))
            # qT = (q_feat + q_pos)^T, host-precomputed in bf16
            qT = cp.tile([D, NQP], BF16, tag="qT")
            nc.sync.dma_start(qT[:], qsT[:])
            # per-tile head-phase results, precomputed for all NT tiles so the
            # head work overlaps the value/table phases
            idx_all = cp.tile([128, NT * NS], INT32, tag="idx_all")
            fxy_all = cp.tile([128, NT * 2 * NS], BF16, tag="fxy_all")
            wy0_all = cp.tile([128, NT * NS], BF16, tag="wy0_all")
            wy1_all = cp.tile([128, NT * NS], BF16, tag="wy1_all")
            # zero the TBL overlap pad so the phase-B overlapped read is defined
            zpad = cp.tile([1, OVER * DH], BF16, tag="zpad")
            nc.vector.memset(zpad[:], 0)
            nc.sync.dma_start(bass.AP(TBL, H * NCELL * DH, [[OVER * DH, 1], [1, OVER * DH]]), zpad[:])
            with tc.tile_pool(name="hp", bufs=1) as hp, \
                 tc.tile_pool(name="qb", bufs=qbb) as qb, \
                 tc.tile_pool(name="qg", bufs=qgb) as qg, \
                 tc.tile_pool(name="qab", bufs=2) as qab, \
                 tc.tile_pool(name="vps", bufs=2, space="PSUM") as vps, \
                 tc.tile_pool(name="qps", bufs=1, space="PSUM") as qps:

                def emit_head(t):
                    tq = t % NT
                    qs = slice(tq * 128, (tq + 1) * 128)
                    # attention weights (softmax over 20 per head); logits are
                    # O(1) so the max-subtraction is unnecessary in fp32
                    ps_aw = qps.tile([128, NS], FP32, tag="ps_aw")
                    nc.tensor.matmul(out=ps_aw[:], lhsT=qT[:, qs], rhs=Wa_sb[:], start=True, stop=True)
                    logit = hp.tile([128, NS], FP32, tag="logit")
                    nc.vector.tensor_tensor(out=logit[:], in0=ps_aw[:], in1=ba_sb[:], op=AX.add)
                    ee = hp.tile([128, NS], FP32, tag="ee")
                    nc.scalar.activation(out=ee[:], in_=logit[:], func=AF.Exp)
                    sm = hp.tile([128, H], FP32, tag="sm")
                    nc.vector.tensor_reduce(out=sm[:], in_=ee[:].rearrange("p (h k) -> p h k", h=H), axis=mybir.AxisListType.X, op=AX.add)
                    rc = hp.tile([128, H], FP32, tag="rc")
                    nc.vector.reciprocal(out=rc[:], in_=sm[:])
                    aw = hp.tile([128, NS], FP32, tag="aw")
                    rcb = rc[:].rearrange("p (h one) -> p h one", one=1).to_broadcast((128, H, L * NP))
                    nc.vector.tensor_tensor(out=aw[:].rearrange("p (h k) -> p h k", h=H), in0=ee[:].rearrange("p (h k) -> p h k", h=H), in1=rcb, op=AX.mult)

                    # sampling offsets
                    ps_off = qps.tile([128, 320], FP32, tag="ps_off")
                    nc.tensor.matmul(out=ps_off[:], lhsT=qT[:, qs], rhs=Wo_sb[:], start=True, stop=True)
                    off = hp.tile([128, 320], FP32, tag="off")
                    nc.vector.tensor_tensor(out=off[:], in0=ps_off[:], in1=bo_sb[:], op=AX.add)

                    reft = hp.tile([128, 2], FP32, tag="reft")
                    nc.sync.dma_start(reft[:], ref[qs, :])
                    refb = reft[:].rearrange("p (two one) -> p two one", one=1).to_broadcast((128, 2, NS))

                    # positions (x in cols 0:NS, y in NS:2NS): ref*W + off + 0.5, clamp [0, W+1]
                    pxy = hp.tile([128, 2 * NS], FP32, tag="pxy")
                    pv = pxy[:].rearrange("p (two s) -> p two s", two=2)
                    offv = off[:].rearrange("p (s two) -> p two s", two=2)
                    twhv = twh_sb[:].rearrange("p (two s) -> p two s", two=2)
                    tcv = tcxy_sb[:].rearrange("p (two s) -> p two s", two=2)
                    nc.vector.tensor_tensor(out=pv, in0=refb, in1=twhv, op=AX.mult)
                    nc.vector.scalar_tensor_tensor(out=pv, in0=offv, scalar=0.5, in1=pv, op0=AX.add, op1=AX.add)
                    nc.vector.scalar_tensor_tensor(out=pv, in0=pv, scalar=0.0, in1=tcv, op0=AX.max, op1=AX.min)

                    pi = hp.tile([128, 2 * NS], INT32, tag="pi")
                    pf = hp.tile([128, 2 * NS], FP32, tag="pf")
                    nc.scalar.activation(out=pi[:], in_=pxy[:], func=AF.Copy)
                    nc.scalar.activation(out=pf[:], in_=pi[:], func=AF.Copy)
                    fxy = fxy_all[:, tq * 2 * NS:(tq + 1) * 2 * NS]
                    nc.vector.tensor_tensor(out=fxy, in0=pxy[:], in1=pf[:], op=AX.subtract)

                    # gather row index: base + y0*187 + x0 (exact in fp32)
                    gfv = hp.tile([128, NS], FP32, tag="gfv")
                    nc.vector.scalar_tensor_tensor(out=gfv[:], in0=pf[:, NS:2 * NS], scalar=float(STRIDE), in1=tbase_sb[:], op0=AX.mult, op1=AX.add)
                    nc.vector.tensor_tensor(out=gfv[:], in0=gfv[:], in1=pf[:, 0:NS], op=AX.add)
                    nc.scalar.activation(out=idx_all[:, tq * NS:(tq + 1) * NS], in_=gfv[:], func=AF.Copy)

                    # y-blend weights
                    wy1 = wy1_all[:, tq * NS:(tq + 1) * NS]
                    wy0 = wy0_all[:, tq * NS:(tq + 1) * NS]
                    nc.vector.tensor_tensor(out=wy1, in0=aw[:], in1=fxy[:, NS:2 * NS], op=AX.mult)
                    nc.vector.tensor_tensor(out=wy0, in0=aw[:], in1=wy1, op=AX.subtract)

                # ---------- value pipeline, head precompute interleaved ----------
                # host permuted vT columns: col g*1024 + j*128 + p <-> cell g*1024 + p*8 + j
                h_emit = 0
                nh0 = min(nt, NT)
                vsteps = 0 if no_vpipe else NGRP // 2
                with tc.tile_pool(name="vload", bufs=3) as vl, \
                     tc.tile_pool(name="vstg", bufs=3) as vstg:
                    for g2 in range(vsteps):     # load 2 groups per DMA
                        vchunk = vl.tile([D + 1, 2048], BF16, tag="vchunk")
                        nc.sync.dma_start(vchunk[:], vT[:, g2 * 2048:(g2 + 1) * 2048])
                        for gg in range(2):
                            g = g2 * 2 + gg
                            ps = vps.tile([128, 512], FP32, tag="vps")
                            for j in range(1 if vmm1 else 8):
                                nc.tensor.matmul(
                                    out=ps[:, j * 64:(j + 1) * 64],
                                    lhsT=vchunk[:, gg * 1024 + j * 128: gg * 1024 + (j + 1) * 128],
                                    rhs=Wv_sb[:],
                                    start=True, stop=True,
                                )
                            # permute (j,h,d) -> (h,j,d) during the PSUM copy so the
                            # per-head store below is contiguous per (head, partition)
                            stg = vstg.tile([128, 512], BF16, tag="vstg")
                            stg_out = stg[:].rearrange("p (h j d) -> p j h d", h=H, j=8, d=DH)
                            ps_in = ps[:].rearrange("p (j h d) -> p j h d", h=H, j=8, d=DH)
                            nc.scalar.activation(out=stg_out, in_=ps_in, func=AF.Copy)
                            # dst elem offset = h*NCELL*8 + g*8192 + p*64 + (j*8+d)
                            dst = bass.AP(TBL, g * 8192,
                                          [[64, 128], [NCELL * DH, H], [1, 64]])
                            nc.sync.dma_start(dst, stg[:].rearrange("p (h f) -> p h f", h=H))
                        while h_emit < (g2 + 1) * nh0 // max(vsteps, 1):
                            emit_head(h_emit)
                            h_emit += 1
                while h_emit < nh0:
                    emit_head(h_emit)
                    h_emit += 1

                # ---------- pair-row table build ----------
                # T3 row r (head h) = [TBL[h, r], TBL[h, r + STRIDE]]
                with tc.tile_pool(name="vc3", bufs=2) as vcp, \
                     tc.tile_pool(name="s3", bufs=1) as s3p:
                    for h in range(0 if (no_vpipe or no_b) else H):
                        vc = vcp.tile([128, (NR + OVER) * DH], BF16, tag="vc")
                        src = bass.AP(TBL, h * NCELL * DH,
                                      [[NR * DH, 128], [1, (NR + OVER) * DH]])
                        nc.sync.dma_start(vc[:], src)
                        stg3 = s3p.tile([128, NR * 2 * DH], BF16, tag="stg3")
                        v3 = stg3[:].rearrange("p (r two d) -> p r two d", two=2, d=DH)
                        vcv = vc[:].rearrange("p (r d) -> p r d", d=DH)
                        nc.scalar.activation(out=v3[:, :, 0, :], in_=vcv[:, 0:NR, :], func=AF.Copy)
                        nc.vector.tensor_scalar(out=v3[:, :, 1, :], in0=vcv[:, STRIDE:STRIDE + NR, :],
                                                scalar1=0.0, scalar2=None, op0=AX.add)
                        dst3 = bass.AP(T3, h * NCELL * 2 * DH,
                                       [[NR * 2 * DH, 128], [1, NR * 2 * DH]])
                        nc.sync.dma_start(dst3, stg3[:])

                # ---------- gather / blend / tail loop ----------
                # tails are batched per NTB tiles; for timing builds with
                # nt > NT, later repeats' heads are emitted one block ahead
                # of their gathers (lookahead software pipeline)
                NTB = 8
                for tb in range(0, nt, NTB):
                    nb = min(NTB, nt - tb)
                    for th in range(max(tb + NTB, nh0), min(tb + 2 * NTB, nt)):
                        emit_head(th)
                    # per-tile 65-wide rows: attn in [0:64], col 64 = 1.0 so the
                    # transposed tile carries a ones-row that folds the bias into
                    # the Wout/W1 matmuls
                    ab = qab.tile([128, (D + 1) * nb], FP32, tag="attnbuf")
                    nc.vector.memset(ab[:].rearrange("p (n e) -> p n e", e=D + 1)[:, :, D:D + 1], 1.0)
                    for t in range(tb, tb + nb):
                        tq = t % NT
                        qs = slice(tq * 128, (tq + 1) * 128)
                        fx = fxy_all[:, tq * 2 * NS:tq * 2 * NS + NS]
                        wy0 = wy0_all[:, tq * NS:(tq + 1) * NS]
                        wy1 = wy1_all[:, tq * NS:(tq + 1) * NS]
                        # gather: one 64B descriptor per slot -> [c00,c10,c01,c11] x 8 dims
                        G = qg.tile([128, NS * 32], BF16, tag="G")
                        if no_gather:
                            nc.vector.memset(G[:], 0)
                        else:
                            for c0 in range(0, NS, gcols):
                                c1 = min(c0 + gcols, NS)
                                nc.gpsimd.indirect_dma_start(
                                    out=G[:, c0 * 32:c1 * 32], out_offset=None,
                                    in_=T3v,
                                    in_offset=bass.IndirectOffsetOnAxis(
                                        ap=idx_all[:, tq * NS + c0:tq * NS + c1], axis=0),
                                )

                        attn = ab[:, (t - tb) * (D + 1):(t - tb) * (D + 1) + D]
                        if no_blend:
                            nc.vector.tensor_reduce(
                                out=attn,
                                in_=G[:].rearrange("p (f r) -> p f r", f=D),
                                axis=mybir.AxisListType.X, op=AX.add)
                        else:
                            # x-lerp then aw-weighted y-blend, in place:
                            #   D = B - A; D *= fx; C = A + D; C0 *= wy0; C1 *= wy1
                            #   m = C0 + C1; attn = sum_lp m
                            Gv = G[:].rearrange("p (s e) -> p s e", e=32)
                            A = Gv[:, :, 0:16]
                            Bv = Gv[:, :, 16:32]
                            Dt = qb.tile([128, NS * 16], BF16, tag="Dt")
                            Dv = Dt[:].rearrange("p (s e) -> p s e", e=16)
                            nc.vector.tensor_tensor(out=Dv, in0=Bv, in1=A, op=AX.subtract)
                            fxb = fx.rearrange("p (s one) -> p s one", one=1).to_broadcast((128, NS, 16))
                            if fxe:
                                # expand fx on ACT so the multiply is fully
                                # packed bf16 (DVE 2x-eligible)
                                fxe_t = qb.tile([128, NS * 16], BF16, tag="fxe")
                                nc.scalar.activation(out=fxe_t[:].rearrange("p (s e) -> p s e", e=16), in_=fxb, func=AF.Copy)
                                nc.vector.tensor_tensor(out=Dv, in0=fxe_t[:].rearrange("p (s e) -> p s e", e=16), in1=Dv, op=AX.mult)
                            else:
                                nc.vector.tensor_tensor(out=Dv, in0=fxb, in1=Dv, op=AX.mult)
                            Ct = qb.tile([128, NS * 16], BF16, tag="Ct")
                            Cv = Ct[:].rearrange("p (s e) -> p s e", e=16)
                            nc.vector.tensor_tensor(out=Cv, in0=A, in1=Dv, op=AX.add)
                            C0 = Cv[:, :, 0:8]
                            C1 = Cv[:, :, 8:16]
                            wy0b = wy0.rearrange("p (s one) -> p s one", one=1).to_broadcast((128, NS, 8))
                            wy1b = wy1.rearrange("p (s one) -> p s one", one=1).to_broadcast((128, NS, 8))
                            nc.vector.tensor_tensor(out=C0, in0=wy0b, in1=C0, op=AX.mult)
                            nc.vector.tensor_tensor(out=C1, in0=wy1b, in1=C1, op=AX.mult)
                            m = Dt[:, 0:NS * 8]
                            nc.vector.tensor_tensor(out=m.rearrange("p (s e) -> p s e", e=8), in0=C0, in1=C1, op=AX.add)
                            nc.vector.tensor_reduce(
                                out=attn.rearrange("p (h d) -> p h d", h=H),
                                in_=m.rearrange("p (h lp d) -> p h d lp", h=H, d=DH),
                                axis=mybir.AxisListType.X, op=AX.add,
                            )

                        if no_tail:
                            nc.sync.dma_start(OUT[qs, :], attn)

                    if no_tail:
                        continue

                    # ---------- batched tails ----------
                    for t in range(tb, tb + nb):
                        tq = t % NT
                        qs = slice(tq * 128, (tq + 1) * 128)
                        a65 = ab[:, (t - tb) * (D + 1):(t - tb + 1) * (D + 1)]
                        # output projection (bias via ones-row) + residual + LN1
                        ps_t = qps.tile([D + 1, 128], FP32, tag="ps_t")
                        nc.tensor.transpose(out=ps_t[:], in_=a65, identity=ident[:])
                        attnT = qb.tile([D + 1, 128], BF16, tag="attnT")
                        nc.scalar.activation(out=attnT[:], in_=ps_t[:], func=AF.Copy)
                        ps_ao = qps.tile([128, D], FP32, tag="ps_ao")
                        nc.tensor.matmul(out=ps_ao[:], lhsT=attnT[:], rhs=Wout_sb[:], start=True, stop=True)
                        qft = qb.tile([128, D], FP32, tag="qft")
                        nc.sync.dma_start(qft[:], qf[qs, :])

                        def layernorm(xin, s1, gg, bb, xout, tag):
                            mn = qb.tile([128, 1], FP32, tag=tag + "_mn")
                            nc.vector.tensor_scalar_mul(out=mn[:], in0=s1[:], scalar1=1.0 / 64.0)
                            xc = qb.tile([128, D], FP32, tag=tag + "_xc")
                            nc.vector.tensor_tensor(out=xc[:], in0=xin, in1=mn[:].to_broadcast((128, D)), op=AX.subtract)
                            sq = qb.tile([128, D], FP32, tag=tag + "_sq")
                            s2 = qb.tile([128, 1], FP32, tag=tag + "_s2")
                            nc.vector.scalar_tensor_tensor(out=sq[:], in0=xc[:], scalar=0.0, in1=xc[:], op0=AX.add, op1=AX.mult, accum_out=s2[:])
                            # rstd = exp(-0.5 * ln(var + eps)); ln and exp share
                            # one ACT func set so no table reloads anywhere
                            lnv = qb.tile([128, 1], FP32, tag=tag + "_lnv")
                            nc.scalar.activation(out=lnv[:], in_=s2[:], func=AF.Ln, scale=1.0 / 64.0, bias=eps_sb[:])
                            rstd = qb.tile([128, 1], FP32, tag=tag + "_rstd")
                            nc.scalar.activation(out=rstd[:], in_=lnv[:], func=AF.Exp, scale=-0.5)
                            nc.vector.scalar_tensor_tensor(out=xout, in0=xc[:], scalar=rstd[:, 0:1], in1=gg[:], op0=AX.mult, op1=AX.mult)
                            nc.vector.tensor_tensor(out=xout, in0=xout, in1=bb[:], op=AX.add)

                        x1e = qb.tile([128, D + 1], FP32, tag="x1e")
                        nc.vector.memset(x1e[:, D:D + 1], 1.0)
                        xpre = qb.tile([128, D], FP32, tag="xpre")
                        s1 = qb.tile([128, 1], FP32, tag="s1")
                        nc.vector.scalar_tensor_tensor(out=xpre[:], in0=ps_ao[:], scalar=0.0, in1=qft[:], op0=AX.add, op1=AX.add, accum_out=s1[:])
                        layernorm(xpre[:], s1, g1_sb, b1_sb, x1e[:, 0:D], "x1")

                        # FFN (W1/Wout carry bias rows; h1 in bf16)
                        ps_t2 = qps.tile([D + 1, 128], FP32, tag="ps_t")
                        nc.tensor.transpose(out=ps_t2[:], in_=x1e[:], identity=ident[:])
                        x1T = qb.tile([D + 1, 128], BF16, tag="x1T")
                        nc.scalar.activation(out=x1T[:], in_=ps_t2[:], func=AF.Copy)
                        h1 = qb.tile([128, 1024], BF16, tag="h1")
                        for k2 in range(2):
                            ps_h1 = qps.tile([128, 512], FP32, tag="ps_h1")
                            for j in range(4):
                                k = k2 * 4 + j
                                nc.tensor.matmul(out=ps_h1[:, j * 128:(j + 1) * 128], lhsT=W1_sb[:, k * 128:(k + 1) * 128], rhs=x1T[:], start=True, stop=True)
                            nc.scalar.activation(out=h1[:, k2 * 512:(k2 + 1) * 512], in_=ps_h1[:], func=AF.Relu)
                        ps_h2 = qps.tile([128, D], FP32, tag="ps_h2")
                        for k in range(8):
                            nc.tensor.matmul(out=ps_h2[:], lhsT=h1[:, k * 128:(k + 1) * 128], rhs=W2_sb[:, k * D:(k + 1) * D], start=(k == 0), stop=(k == 7))
                        bx = qb.tile([128, D], FP32, tag="bx")
                        nc.vector.tensor_tensor(out=bx[:], in0=x1e[:, 0:D], in1=bff2_sb[:], op=AX.add)
                        x2p = qb.tile([128, D], FP32, tag="x2p")
                        s1b = qb.tile([128, 1], FP32, tag="s1b")
                        nc.vector.scalar_tensor_tensor(out=x2p[:], in0=ps_h2[:], scalar=0.0, in1=bx[:], op0=AX.add, op1=AX.add, accum_out=s1b[:])
                        x2 = qb.tile([128, D], FP32, tag="x2")
                        layernorm(x2p[:], s1b, g2_sb, b2_sb, x2[:], "x2")
                        nc.sync.dma_start(OUT[qs, :], x2[:])

    nc.finalize()
    return nc


# revision 5
# speedup vs baseline: 1.0909x; 1.0909x over previous
import sys
sys.path.insert(0, '/opt/trn_rl_repo')
import numpy as np
import ml_dtypes

from concourse import bass, mybir, bacc
from concourse.tile import TileContext
from concourse.masks import make_identity
from concourse import bass_utils

# ---- problem constants (hardcoded) ----
D = 64
H = 8
L = 5
NP = 4            # points
DH = 8
NQ = 20000
B = 2
LIN = 45109
SS = [(184, 184), (92, 92), (46, 46), (23, 23), (12, 12)]   # (Hl, Wl)
LSI = [0, 33856, 42320, 44436, 44965]
STRIDE = 187                       # padded row stride in cells (>= Wl+3)
ROWS = [h + 3 for (h, w) in SS]    # rows incl. 1-top border + 2 bottom pad
CL = [r * STRIDE for r in ROWS]    # raw cells per level
CLP = [((c + 1023) // 1024) * 1024 for c in CL]   # level cells padded to groups
LBASE = [0]
for c in CLP[:-1]:
    LBASE.append(LBASE[-1] + c)
NCELL = sum(CLP)                   # 71680
NGRP = NCELL // 1024               # 70
NR = NCELL // 128                  # 560 rows per partition per head
OVER = 188                         # pair-row overlap read per partition
NQP = 5120                         # queries per core (padded)
NT = NQP // 128                    # 40 query tiles
NS = H * L * NP                    # 160 sample slots per query
FP32 = mybir.dt.float32
BF16 = mybir.dt.bfloat16
INT32 = mybir.dt.int32
AX = mybir.AluOpType
AF = mybir.ActivationFunctionType


def _build_tables():
    # per-slot (j = h*20 + l*4 + p) constant rows
    t_wl = np.zeros(NS, np.float32)
    t_hl = np.zeros(NS, np.float32)
    t_cxhi = np.zeros(NS, np.float32)
    t_cyhi = np.zeros(NS, np.float32)
    t_base = np.zeros(NS, np.float32)
    for h in range(H):
        for l in range(L):
            hl, wl = SS[l]
            for p in range(NP):
                j = h * (L * NP) + l * NP + p
                t_wl[j] = wl
                t_hl[j] = hl
                t_cxhi[j] = wl + 1
                t_cyhi[j] = hl + 1
                t_base[j] = (h % 4) * NCELL + LBASE[l]
    return t_wl, t_hl, t_cxhi, t_cyhi, t_base


def build_program(nt=NT, gcols=80, no_gather=False, no_vpipe=False,
                  no_blend=False, no_tail=False, no_b=False, vmm1=False,
                  fxe=False, qhb=2, qbb=2, qgb=2):
    nc = bacc.Bacc()
    dt = nc.dram_tensor
    vT = dt("vT", (D + 1, NCELL), BF16, kind="ExternalInput")
    qsT = dt("qsT", (D, NQP), BF16, kind="ExternalInput")
    qf = dt("qf", (NQP, D), FP32, kind="ExternalInput")
    ref = dt("ref", (NQP, 2), FP32, kind="ExternalInput")
    Wv = dt("Wv", (D + 1, D), BF16, kind="ExternalInput")
    Wo = dt("Wo", (D, H * L * NP * 2), BF16, kind="ExternalInput")
    Wa = dt("Wa", (D, NS), BF16, kind="ExternalInput")
    Wout = dt("Wout", (D + 1, D), BF16, kind="ExternalInput")
    W1 = dt("W1", (D + 1, 1024), BF16, kind="ExternalInput")
    W2 = dt("W2", (128, 8 * D), BF16, kind="ExternalInput")
    bo_r = dt("bo_r", (128, 320), FP32, kind="ExternalInput")
    ba_r = dt("ba_r", (128, NS), FP32, kind="ExternalInput")
    g1_r = dt("g1_r", (128, D), FP32, kind="ExternalInput")
    b1_r = dt("b1_r", (128, D), FP32, kind="ExternalInput")
    g2_r = dt("g2_r", (128, D), FP32, kind="ExternalInput")
    b2_r = dt("b2_r", (128, D), FP32, kind="ExternalInput")
    bff2_r = dt("bff2_r", (128, D), FP32, kind="ExternalInput")
    t_wh = dt("t_wh", (128, 2 * NS), FP32, kind="ExternalInput")
    t_cxy = dt("t_cxy", (128, 2 * NS), FP32, kind="ExternalInput")
    t_base = dt("t_base", (128, NS), FP32, kind="ExternalInput")
    OUT = dt("out", (NQP, D), FP32, kind="ExternalOutput")
    # per-head table: row (h*NCELL + cell) holds that head's 8 dims of a cell
    TBL = dt("tbl", ((H * NCELL + OVER) * DH,), BF16, kind="Internal")
    # pair-row table: row ((h%4)*NCELL + cell) holds 8 dims of cell and
    # cell+STRIDE; split into two 4-head tensors so the first gather call
    # (heads 0-3) only depends on half of the table build
    T3a = dt("t3a", (4 * NCELL * 2 * DH,), BF16, kind="Internal")
    T3b = dt("t3b", (4 * NCELL * 2 * DH,), BF16, kind="Internal")
    T3av = T3a[:].rearrange("(c d) -> c d", d=2 * DH)
    T3bv = T3b[:].rearrange("(c d) -> c d", d=2 * DH)

    with TileContext(nc) as tc:
        with tc.tile_pool(name="const", bufs=1) as cp:
            def ld(src, shape, dtype=FP32):
                t = cp.tile(shape, dtype, tag=src.name + "_sb")
                nc.sync.dma_start(t[:], src[:])
                return t
            Wv_sb = ld(Wv, [D + 1, D], BF16)
            Wo_sb = ld(Wo, [D, 320], BF16)
            Wa_sb = ld(Wa, [D, NS], BF16)
            Wout_sb = ld(Wout, [D + 1, D], BF16)
            W1_sb = ld(W1, [D + 1, 1024], BF16)
            W2_sb = ld(W2, [128, 8 * D], BF16)
            bo_sb = ld(bo_r, [128, 320])
            ba_sb = ld(ba_r, [128, NS])
            g1_sb = ld(g1_r, [128, D])
            b1_sb = ld(b1_r, [128, D])
            g2_sb = ld(g2_r, [128, D])
            b2_sb = ld(b2_r, [128, D])
            bff2_sb = ld(bff2_r, [128, D])
            twh_sb = ld(t_wh, [128, 2 * NS])
            tcxy_sb = ld(t_cxy, [128, 2 * NS])
            tbase_sb = ld(t_base, [128, NS])
            eps_sb = cp.tile([128, 1], FP32, tag="eps")
            nc.vector.memset(eps_sb[:], 1e-5)
            ident = cp.tile([128, 128], FP32, tag="ident")
            make_identity(nc, ident[:])
            # preload the one ACT func set that covers every func we use
            # (copy/exp/ln/relu: natural_log_exp_and_others, id 6) so the
            # table-load pass never inserts another reload
            nc.scalar.add_instruction(mybir.InstLoadActFuncSet(
                name=nc.scalar.bass.get_next_instruction_name(),
                act_func_set_id=6, ins=[], outs=[]))
            # qT = (q_feat + q_pos)^T, host-precomputed in bf16
            qT = cp.tile([D, NQP], BF16, tag="qT")
            nc.sync.dma_start(qT[:], qsT[:])
            # per-tile head-phase results, precomputed for all NT tiles so the
            # head work overlaps the value/table phases
            idx_all = cp.tile([128, NT * NS], INT32, tag="idx_all")
            fxy_all = cp.tile([128, NT * 2 * NS], BF16, tag="fxy_all")
            wy0_all = cp.tile([128, NT * NS], BF16, tag="wy0_all")
            wy1_all = cp.tile([128, NT * NS], BF16, tag="wy1_all")
            # zero the TBL overlap pad so the phase-B overlapped read is defined
            zpad = cp.tile([1, OVER * DH], BF16, tag="zpad")
            nc.vector.memset(zpad[:], 0)
            nc.sync.dma_start(bass.AP(TBL, H * NCELL * DH, [[OVER * DH, 1], [1, OVER * DH]]), zpad[:])
            with tc.tile_pool(name="hp", bufs=1) as hp, \
                 tc.tile_pool(name="qb", bufs=qbb) as qb, \
                 tc.tile_pool(name="qg", bufs=qgb) as qg, \
                 tc.tile_pool(name="qab", bufs=2) as qab, \
                 tc.tile_pool(name="vps", bufs=2, space="PSUM") as vps, \
                 tc.tile_pool(name="qps", bufs=1, space="PSUM") as qps:

                def emit_head(t):
                    tq = t % NT
                    qs = slice(tq * 128, (tq + 1) * 128)
                    # attention weights (softmax over 20 per head); logits are
                    # O(1) so the max-subtraction is unnecessary in fp32
                    ps_aw = qps.tile([128, NS], FP32, tag="ps_aw")
                    nc.tensor.matmul(out=ps_aw[:], lhsT=qT[:, qs], rhs=Wa_sb[:], start=True, stop=True)
                    logit = hp.tile([128, NS], FP32, tag="logit")
                    nc.vector.tensor_tensor(out=logit[:], in0=ps_aw[:], in1=ba_sb[:], op=AX.add)
                    ee = hp.tile([128, NS], FP32, tag="ee")
                    nc.scalar.activation(out=ee[:], in_=logit[:], func=AF.Exp)
                    sm = hp.tile([128, H], FP32, tag="sm")
                    nc.vector.tensor_reduce(out=sm[:], in_=ee[:].rearrange("p (h k) -> p h k", h=H), axis=mybir.AxisListType.X, op=AX.add)
                    rc = hp.tile([128, H], FP32, tag="rc")
                    nc.vector.reciprocal(out=rc[:], in_=sm[:])
                    aw = hp.tile([128, NS], FP32, tag="aw")
                    rcb = rc[:].rearrange("p (h one) -> p h one", one=1).to_broadcast((128, H, L * NP))
                    nc.vector.tensor_tensor(out=aw[:].rearrange("p (h k) -> p h k", h=H), in0=ee[:].rearrange("p (h k) -> p h k", h=H), in1=rcb, op=AX.mult)

                    # sampling offsets
                    ps_off = qps.tile([128, 320], FP32, tag="ps_off")
                    nc.tensor.matmul(out=ps_off[:], lhsT=qT[:, qs], rhs=Wo_sb[:], start=True, stop=True)
                    off = hp.tile([128, 320], FP32, tag="off")
                    nc.vector.tensor_tensor(out=off[:], in0=ps_off[:], in1=bo_sb[:], op=AX.add)

                    reft = hp.tile([128, 2], FP32, tag="reft")
                    nc.sync.dma_start(reft[:], ref[qs, :])
                    refb = reft[:].rearrange("p (two one) -> p two one", one=1).to_broadcast((128, 2, NS))

                    # positions (x in cols 0:NS, y in NS:2NS): ref*W + off + 0.5, clamp [0, W+1]
                    pxy = hp.tile([128, 2 * NS], FP32, tag="pxy")
                    pv = pxy[:].rearrange("p (two s) -> p two s", two=2)
                    offv = off[:].rearrange("p (s two) -> p two s", two=2)
                    twhv = twh_sb[:].rearrange("p (two s) -> p two s", two=2)
                    tcv = tcxy_sb[:].rearrange("p (two s) -> p two s", two=2)
                    nc.vector.tensor_tensor(out=pv, in0=refb, in1=twhv, op=AX.mult)
                    nc.vector.scalar_tensor_tensor(out=pv, in0=offv, scalar=0.5, in1=pv, op0=AX.add, op1=AX.add)
                    nc.vector.scalar_tensor_tensor(out=pv, in0=pv, scalar=0.0, in1=tcv, op0=AX.max, op1=AX.min)

                    pi = hp.tile([128, 2 * NS], INT32, tag="pi")
                    pf = hp.tile([128, 2 * NS], FP32, tag="pf")
                    nc.scalar.activation(out=pi[:], in_=pxy[:], func=AF.Copy)
                    nc.scalar.activation(out=pf[:], in_=pi[:], func=AF.Copy)
                    fxy = fxy_all[:, tq * 2 * NS:(tq + 1) * 2 * NS]
                    nc.vector.tensor_tensor(out=fxy, in0=pxy[:], in1=pf[:], op=AX.subtract)

                    # gather row index: base + y0*187 + x0 (exact in fp32)
                    gfv = hp.tile([128, NS], FP32, tag="gfv")
                    nc.vector.scalar_tensor_tensor(out=gfv[:], in0=pf[:, NS:2 * NS], scalar=float(STRIDE), in1=tbase_sb[:], op0=AX.mult, op1=AX.add)
                    nc.vector.tensor_tensor(out=gfv[:], in0=gfv[:], in1=pf[:, 0:NS], op=AX.add)
                    nc.scalar.activation(out=idx_all[:, tq * NS:(tq + 1) * NS], in_=gfv[:], func=AF.Copy)

                    # y-blend weights
                    wy1 = wy1_all[:, tq * NS:(tq + 1) * NS]
                    wy0 = wy0_all[:, tq * NS:(tq + 1) * NS]
                    nc.vector.tensor_tensor(out=wy1, in0=aw[:], in1=fxy[:, NS:2 * NS], op=AX.mult)
                    nc.vector.tensor_tensor(out=wy0, in0=aw[:], in1=wy1, op=AX.subtract)

                # ---------- value pipeline, head precompute interleaved ----------
                # host permuted vT columns: col g*1024 + j*128 + p <-> cell g*1024 + p*8 + j
                h_emit = 0
                nh0 = min(nt, NT)
                vsteps = 0 if no_vpipe else NGRP // 2
                with tc.tile_pool(name="vload", bufs=3) as vl, \
                     tc.tile_pool(name="vstg", bufs=3) as vstg:
                    for g2 in range(vsteps):     # load 2 groups per DMA
                        vchunk = vl.tile([D + 1, 2048], BF16, tag="vchunk")
                        nc.sync.dma_start(vchunk[:], vT[:, g2 * 2048:(g2 + 1) * 2048])
                        for gg in range(2):
                            g = g2 * 2 + gg
                            ps = vps.tile([128, 512], FP32, tag="vps")
                            for j in range(1 if vmm1 else 8):
                                nc.tensor.matmul(
                                    out=ps[:, j * 64:(j + 1) * 64],
                                    lhsT=vchunk[:, gg * 1024 + j * 128: gg * 1024 + (j + 1) * 128],
                                    rhs=Wv_sb[:],
                                    start=True, stop=True,
                                )
                            # permute (j,h,d) -> (h,j,d) during the PSUM copy so the
                            # per-head store below is contiguous per (head, partition)
                            stg = vstg.tile([128, 512], BF16, tag="vstg")
                            stg_out = stg[:].rearrange("p (h j d) -> p j h d", h=H, j=8, d=DH)
                            ps_in = ps[:].rearrange("p (j h d) -> p j h d", h=H, j=8, d=DH)
                            nc.scalar.activation(out=stg_out, in_=ps_in, func=AF.Copy)
                            # dst elem offset = h*NCELL*8 + g*8192 + p*64 + (j*8+d)
                            dst = bass.AP(TBL, g * 8192,
                                          [[64, 128], [NCELL * DH, H], [1, 64]])
                            nc.sync.dma_start(dst, stg[:].rearrange("p (h f) -> p h f", h=H))
                        while h_emit < (g2 + 1) * nh0 // max(vsteps, 1):
                            emit_head(h_emit)
                            h_emit += 1
                while h_emit < nh0:
                    emit_head(h_emit)
                    h_emit += 1

                # ---------- pair-row table build ----------
                # T3 row r (head h) = [TBL[h, r], TBL[h, r + STRIDE]]
                with tc.tile_pool(name="vc3", bufs=2) as vcp, \
                     tc.tile_pool(name="s3", bufs=1) as s3p:
                    for h in range(0 if (no_vpipe or no_b) else H):
                        vc = vcp.tile([128, (NR + OVER) * DH], BF16, tag="vc")
                        src = bass.AP(TBL, h * NCELL * DH,
                                      [[NR * DH, 128], [1, (NR + OVER) * DH]])
                        nc.sync.dma_start(vc[:], src)
                        stg3 = s3p.tile([128, NR * 2 * DH], BF16, tag="stg3")
                        v3 = stg3[:].rearrange("p (r two d) -> p r two d", two=2, d=DH)
                        vcv = vc[:].rearrange("p (r d) -> p r d", d=DH)
                        nc.scalar.activation(out=v3[:, :, 0, :], in_=vcv[:, 0:NR, :], func=AF.Copy)
                        nc.vector.tensor_scalar(out=v3[:, :, 1, :], in0=vcv[:, STRIDE:STRIDE + NR, :],
                                                scalar1=0.0, scalar2=None, op0=AX.add)
                        dst3 = bass.AP(T3a if h < 4 else T3b, (h % 4) * NCELL * 2 * DH,
                                       [[NR * 2 * DH, 128], [1, NR * 2 * DH]])
                        nc.sync.dma_start(dst3, stg3[:])

                # ---------- gather / blend / tail loop ----------
                # tails are batched per NTB tiles; for timing builds with
                # nt > NT, later repeats' heads are emitted one block ahead
                # of their gathers (lookahead software pipeline)
                NTB = 8
                for tb in range(0, nt, NTB):
                    nb = min(NTB, nt - tb)
                    for th in range(max(tb + NTB, nh0), min(tb + 2 * NTB, nt)):
                        emit_head(th)
                    # per-tile 65-wide rows: attn in [0:64], col 64 = 1.0 so the
                    # transposed tile carries a ones-row that folds the bias into
                    # the Wout/W1 matmuls
                    ab = qab.tile([128, (D + 1) * nb], FP32, tag="attnbuf")
                    nc.vector.memset(ab[:].rearrange("p (n e) -> p n e", e=D + 1)[:, :, D:D + 1], 1.0)
                    for t in range(tb, tb + nb):
                        tq = t % NT
                        qs = slice(tq * 128, (tq + 1) * 128)
                        fx = fxy_all[:, tq * 2 * NS:tq * 2 * NS + NS]
                        wy0 = wy0_all[:, tq * NS:(tq + 1) * NS]
                        wy1 = wy1_all[:, tq * NS:(tq + 1) * NS]
                        # gather: one 64B descriptor per slot -> [c00,c10,c01,c11] x 8 dims
                        G = qg.tile([128, NS * 32], BF16, tag="G")
                        if no_gather:
                            nc.vector.memset(G[:], 0)
                        else:
                            for c0 in range(0, NS, 80):
                                c1 = c0 + 80
                                nc.gpsimd.indirect_dma_start(
                                    out=G[:, c0 * 32:c1 * 32], out_offset=None,
                                    in_=T3av if c0 == 0 else T3bv,
                                    in_offset=bass.IndirectOffsetOnAxis(
                                        ap=idx_all[:, tq * NS + c0:tq * NS + c1], axis=0),
                                )

                        attn = ab[:, (t - tb) * (D + 1):(t - tb) * (D + 1) + D]
                        if no_blend:
                            nc.vector.tensor_reduce(
                                out=attn,
                                in_=G[:].rearrange("p (f r) -> p f r", f=D),
                                axis=mybir.AxisListType.X, op=AX.add)
                        else:
                            # x-lerp then aw-weighted y-blend, in place:
                            #   D = B - A; D *= fx; C = A + D; C0 *= wy0; C1 *= wy1
                            #   m = C0 + C1; attn = sum_lp m
                            Gv = G[:].rearrange("p (s e) -> p s e", e=32)
                            A = Gv[:, :, 0:16]
                            Bv = Gv[:, :, 16:32]
                            Dt = qb.tile([128, NS * 16], BF16, tag="Dt")
                            Dv = Dt[:].rearrange("p (s e) -> p s e", e=16)
                            nc.vector.tensor_tensor(out=Dv, in0=Bv, in1=A, op=AX.subtract)
                            fxb = fx.rearrange("p (s one) -> p s one", one=1).to_broadcast((128, NS, 16))
                            if fxe:
                                # expand fx on ACT so the multiply is fully
                                # packed bf16 (DVE 2x-eligible)
                                fxe_t = qb.tile([128, NS * 16], BF16, tag="fxe")
                                nc.scalar.activation(out=fxe_t[:].rearrange("p (s e) -> p s e", e=16), in_=fxb, func=AF.Copy)
                                nc.vector.tensor_tensor(out=Dv, in0=fxe_t[:].rearrange("p (s e) -> p s e", e=16), in1=Dv, op=AX.mult)
                            else:
                                nc.vector.tensor_tensor(out=Dv, in0=fxb, in1=Dv, op=AX.mult)
                            Ct = qb.tile([128, NS * 16], BF16, tag="Ct")
                            Cv = Ct[:].rearrange("p (s e) -> p s e", e=16)
                            nc.vector.tensor_tensor(out=Cv, in0=A, in1=Dv, op=AX.add)
                            C0 = Cv[:, :, 0:8]
                            C1 = Cv[:, :, 8:16]
                            wy0b = wy0.rearrange("p (s one) -> p s one", one=1).to_broadcast((128, NS, 8))
                            wy1b = wy1.rearrange("p (s one) -> p s one", one=1).to_broadcast((128, NS, 8))
                            nc.vector.tensor_tensor(out=C0, in0=wy0b, in1=C0, op=AX.mult)
                            nc.vector.tensor_tensor(out=C1, in0=wy1b, in1=C1, op=AX.mult)
                            m = Dt[:, 0:NS * 8]
                            nc.vector.tensor_tensor(out=m.rearrange("p (s e) -> p s e", e=8), in0=C0, in1=C1, op=AX.add)
                            nc.vector.tensor_reduce(
                                out=attn.rearrange("p (h d) -> p h d", h=H),
                                in_=m.rearrange("p (h lp d) -> p h d lp", h=H, d=DH),
                                axis=mybir.AxisListType.X, op=AX.add,
                            )

                        if no_tail:
                            nc.sync.dma_start(OUT[qs, :], attn)

                    if no_tail:
                        continue

                    # ---------- batched tails ----------
                    for t in range(tb, tb + nb):
                        tq = t % NT
                        qs = slice(tq * 128, (tq + 1) * 128)
                        a65 = ab[:, (t - tb) * (D + 1):(t - tb + 1) * (D + 1)]
                        # output projection (bias via ones-row) + residual + LN1
                        ps_t = qps.tile([D + 1, 128], FP32, tag="ps_t")
                        nc.tensor.transpose(out=ps_t[:], in_=a65, identity=ident[:])
                        attnT = qb.tile([D + 1, 128], BF16, tag="attnT")
                        nc.scalar.activation(out=attnT[:], in_=ps_t[:], func=AF.Copy)
                        ps_ao = qps.tile([128, D], FP32, tag="ps_ao")
                        nc.tensor.matmul(out=ps_ao[:], lhsT=attnT[:], rhs=Wout_sb[:], start=True, stop=True)
                        qft = qb.tile([128, D], FP32, tag="qft")
                        nc.sync.dma_start(qft[:], qf[qs, :])

                        def layernorm(xin, s1, gg, bb, xout, tag):
                            mn = qb.tile([128, 1], FP32, tag=tag + "_mn")
                            nc.vector.tensor_scalar_mul(out=mn[:], in0=s1[:], scalar1=1.0 / 64.0)
                            xc = qb.tile([128, D], FP32, tag=tag + "_xc")
                            nc.vector.tensor_tensor(out=xc[:], in0=xin, in1=mn[:].to_broadcast((128, D)), op=AX.subtract)
                            sq = qb.tile([128, D], FP32, tag=tag + "_sq")
                            s2 = qb.tile([128, 1], FP32, tag=tag + "_s2")
                            nc.vector.scalar_tensor_tensor(out=sq[:], in0=xc[:], scalar=0.0, in1=xc[:], op0=AX.add, op1=AX.mult, accum_out=s2[:])
                            # rstd = exp(-0.5 * ln(var + eps)); ln and exp share
                            # one ACT func set so no table reloads anywhere
                            lnv = qb.tile([128, 1], FP32, tag=tag + "_lnv")
                            nc.scalar.activation(out=lnv[:], in_=s2[:], func=AF.Ln, scale=1.0 / 64.0, bias=eps_sb[:])
                            rstd = qb.tile([128, 1], FP32, tag=tag + "_rstd")
                            nc.scalar.activation(out=rstd[:], in_=lnv[:], func=AF.Exp, scale=-0.5)
                            nc.vector.scalar_tensor_tensor(out=xout, in0=xc[:], scalar=rstd[:, 0:1], in1=gg[:], op0=AX.mult, op1=AX.mult)
                            nc.vector.tensor_tensor(out=xout, in0=xout, in1=bb[:], op=AX.add)

                        x1e = qb.tile([128, D + 1], FP32, tag="x1e")
                        nc.vector.memset(x1e[:, D:D + 1], 1.0)
                        xpre = qb.tile([128, D], FP32, tag="xpre")
                        s1 = qb.tile([128, 1], FP32, tag="s1")
                        nc.vector.scalar_tensor_tensor(out=xpre[:], in0=ps_ao[:], scalar=0.0, in1=qft[:], op0=AX.add, op1=AX.add, accum_out=s1[:])
                        layernorm(xpre[:], s1, g1_sb, b1_sb, x1e[:, 0:D], "x1")

                        # FFN (W1/Wout carry bias rows; h1 in bf16)
                        ps_t2 = qps.tile([D + 1, 128], FP32, tag="ps_t")
                        nc.tensor.transpose(out=ps_t2[:], in_=x1e[:], identity=ident[:])
                        x1T = qb.tile([D + 1, 128], BF16, tag="x1T")
                        nc.scalar.activation(out=x1T[:], in_=ps_t2[:], func=AF.Copy)
                        h1 = qb.tile([128, 1024], BF16, tag="h1")
                        for k2 in range(2):
                            ps_h1 = qps.tile([128, 512], FP32, tag="ps_h1")
                            for j in range(4):
                                k = k2 * 4 + j
                                nc.tensor.matmul(out=ps_h1[:, j * 128:(j + 1) * 128], lhsT=W1_sb[:, k * 128:(k + 1) * 128], rhs=x1T[:], start=True, stop=True)
                            nc.scalar.activation(out=h1[:, k2 * 512:(k2 + 1) * 512], in_=ps_h1[:], func=AF.Relu)
                        ps_h2 = qps.tile([128, D], FP32, tag="ps_h2")
                        for k in range(8):
                            nc.tensor.matmul(out=ps_h2[:], lhsT=h1[:, k * 128:(k + 1) * 128], rhs=W2_sb[:, k * D:(k + 1) * D], start=(k == 0), stop=(k == 7))
                        bx = qb.tile([128, D], FP32, tag="bx")
                        nc.vector.tensor_tensor(out=bx[:], in0=x1e[:, 0:D], in1=bff2_sb[:], op=AX.add)
                        x2p = qb.tile([128, D], FP32, tag="x2p")
                        s1b = qb.tile([128, 1], FP32, tag="s1b")
                        nc.vector.scalar_tensor_tensor(out=x2p[:], in0=ps_h2[:], scalar=0.0, in1=bx[:], op0=AX.add, op1=AX.add, accum_out=s1b[:])
                        x2 = qb.tile([128, D], FP32, tag="x2")
                        layernorm(x2p[:], s1b, g2_sb, b2_sb, x2[:], "x2")
                        nc.sync.dma_start(OUT[qs, :], x2[:])

    nc.finalize()
    return nc
